# revision 22
# baseline (speedup 1.0000x reference)
"""CondConv (routing -> per-sample mixed 3x3 conv -> frozen BN -> ReLU -> residual)
on 8 Trainium2 NeuronCores, data-parallel over batch (4 samples/core).

Per core:
  - expert bank resident in SBUF as bf16, host-pretransposed to
    [cin, cout-half, kk, 128] so each cout half is contiguous
  - routing: GAP (DVE reduce) -> dot with route_w (DVE + gpsimd partition
    all-reduce; keeps the PE queue free for conv matmuls) -> sigmoid (ACT)
  - per-sample mixed kernel: DVE scalar_tensor_tensor accumulation in bf16,
    split per cout half so the first conv starts after half the mixing
  - conv: per output tile, 18 accumulating bf16 matmuls (2 cin tiles x 3x3
    taps; fp32 PSUM) against width-padded bf16 images; moving dim = 8 rows
    x 56 cols = 448; bf16 weight loads get FWL so LDW hides under the stream
  - BN(frozen)+ReLU fused into the ACT PSUM evacuation, residual add on DVE,
    fp32 output
"""

import threading

import ml_dtypes
import numpy as np

import concourse.bass as bass
import concourse.mybir as mybir
import concourse.tile as tile
from concourse import bacc, bass_isa
from concourse.bass_utils import run_bass_kernel_spmd

F32 = mybir.dt.float32
BF16 = mybir.dt.bfloat16
FP8 = mybir.dt.float8e4
DR = mybir.MatmulPerfMode.DoubleRow
AX = mybir.AxisListType
OP = mybir.AluOpType
AF = mybir.ActivationFunctionType

N_CORES = 8
B, CIN, COUT, H, W, KS, E = 32, 256, 256, 56, 56, 3, 4
BPC = B // N_CORES  # samples per core
CT = CIN // 128     # cin partition tiles
OTN = COUT // 128   # cout partition tiles
KK = KS * KS
WP = W + 2          # width zero-padded (kj shifts); height handled by clipping
RC = 7              # row chunks per image
RH = H // RC        # rows per chunk
NF = RH * W         # moving-dim elements per matmul
BN_EPS = 1e-5
WSC = 64.0          # weight pre-scale for e4m3 range; 1/WSC on ACT evacuation

# tap split: center+corners run as fp8e4 DoubleRow matmuls (256-deep cin
# contraction, ~1.44x PE throughput); edges stay bf16. 5/9 of the contraction
# in fp8 keeps the measured pipeline rel err at ~1.8e-2 (< 2e-2 budget).
# Center first: it covers the full output chunk, so it carries start=True and
# clears every PSUM has_written bit; row-clipped taps then accumulate flat
# sub-slices (= 'same' padding semantics at top/bottom).
FP8_TAPS = [(1, 1), (0, 0), (0, 2), (2, 0), (2, 2), (0, 1), (1, 0)]
BF16_TAPS = [(1, 2), (2, 1)]
NQ = len(FP8_TAPS)
NB = len(BF16_TAPS)


def build_bass():
    nc = bacc.Bacc("TRN2", target_bir_lowering=False, debug=False)

    x_d = nc.dram_tensor("x", [BPC, CIN, H, WP], BF16, kind="ExternalInput")
    xq_d = nc.dram_tensor("xq", [BPC, 128, CT, H, WP], FP8,
                          kind="ExternalInput")
    wt_d = nc.dram_tensor("wt", [E, CIN, OTN, NB, 128], BF16,
                          kind="ExternalInput")
    wq_d = nc.dram_tensor("wq", [E, 128, OTN, NQ, CT, 128], BF16,
                          kind="ExternalInput")
    rwt_d = nc.dram_tensor("rwt", [CIN, E], F32, kind="ExternalInput")
    rb_d = nc.dram_tensor("rb", [E], F32, kind="ExternalInput")
    bnb_d = nc.dram_tensor("bnb", [128, OTN], F32, kind="ExternalInput")
    y_d = nc.dram_tensor("y", [BPC, COUT, H, W], BF16, kind="ExternalOutput")

    x_ap = x_d.ap()
    xq_ap = xq_d.ap()
    wq_ap = wq_d.ap()
    wt_ap = wt_d.ap()
    rwt_ap = rwt_d.ap()
    rb_ap = rb_d.ap()
    bnb_ap = bnb_d.ap()
    y_ap = y_d.ap()

    with tile.TileContext(nc) as tc:
        with (
            tc.tile_pool(name="wbp", bufs=1) as wbp,
            tc.tile_pool(name="xpp", bufs=1) as xpp,
            tc.tile_pool(name="mwp", bufs=1) as mwp,
            tc.tile_pool(name="otp", bufs=10) as otp,
            tc.tile_pool(name="snp", bufs=1) as snp,
            tc.tile_pool(name="smp", bufs=4) as smp,
            tc.tile_pool(name="psp", bufs=6, space="PSUM") as psp,
        ):
            # ---- persistent tiles ----
            wb = [[wbp.tile([128, OTN, NB, 128], BF16, name=f"wb{e}_{t}",
                            tag=f"wb{e}_{t}")
                   for t in range(CT)] for e in range(E)]
            wq = [wbp.tile([128, OTN, NQ, CT, 128], BF16, name=f"wq{e}",
                           tag=f"wq{e}") for e in range(E)]
            xp = [[xpp.tile([128, H, WP], BF16, name=f"xp{i}_{t}",
                            tag=f"xp{i}_{t}")
                   for t in range(CT)] for i in range(2)]
            xq = [xpp.tile([128, CT, H, WP], FP8, name=f"xq{i}", tag=f"xq{i}")
                  for i in range(2)]
            mw = [[mwp.tile([128, OTN, NB, 128], BF16, name=f"mw{i}_{t}",
                            tag=f"mw{i}_{t}")
                   for t in range(CT)] for i in range(2)]
            mwq = [mwp.tile([128, OTN, NQ, CT, 128], FP8, name=f"mwq{i}",
                            tag=f"mwq{i}") for i in range(2)]
            rwt_sb = [snp.tile([128, E], F32, name=f"rwt{t}", tag=f"rwt{t}")
                      for t in range(CT)]
            rb_bc = snp.tile([128, E], F32, name="rb_bc", tag="rb_bc")
            bnb_sb = snp.tile([128, OTN], F32, name="bnb", tag="bnb")

            # ---- preamble DMAs in priority order: queue order = bandwidth
            # priority. x(0) tiles split across two queues (routing critical
            # path), tiny params next, then the expert bank with the oi=0
            # halves first (mixing consumes them first).
            nc.sync.dma_start(out=xp[0][0][:, 0:28], in_=x_ap[0, 0:128, 0:28, :])
            nc.scalar.dma_start(out=xp[0][0][:, 28:56], in_=x_ap[0, 0:128, 28:56, :])
            nc.sync.dma_start(out=xp[0][1][:, 0:28], in_=x_ap[0, 128:256, 0:28, :])
            nc.gpsimd.dma_start(out=xp[0][1][:, 28:56], in_=x_ap[0, 128:256, 28:56, :])
            nc.sync.dma_start(out=xq[0][:, :, 0:28], in_=xq_ap[0, :, :, 0:28])
            nc.scalar.dma_start(out=xq[0][:, :, 28:56], in_=xq_ap[0, :, :, 28:56])
            for t in range(CT):
                nc.sync.dma_start(out=rwt_sb[t],
                                  in_=rwt_ap[t * 128:(t + 1) * 128, :])
            nc.sync.dma_start(
                out=rb_bc,
                in_=bass.AP(tensor=rb_ap.tensor, offset=0, ap=[[0, 128], [1, E]]))
            nc.sync.dma_start(out=bnb_sb, in_=bnb_ap[:, :])
            for e in range(E):
                nc.sync.dma_start(out=wq[e], in_=wq_ap[e])
            for oi in range(OTN):
                for e in range(E):
                    for t in range(CT):
                        nc.sync.dma_start(out=wb[e][t][:, oi],
                                          in_=wt_ap[e, t * 128:(t + 1) * 128, oi])

            # all-zeros per-partition scalar: explicit AP bias for ACT funcs
            # (the float-bias path needs a pre-registered const-AP database)
            zeros1 = snp.tile([128, 1], F32, name="zeros1", tag="zeros1")
            nc.vector.memset(zeros1, 0.0)
            sc64 = snp.tile([128, 1], F32, name="sc64", tag="sc64")
            nc.vector.memset(sc64, 1.0 / WSC)
            mxc = [snp.tile([128, NQ * CT * 128], BF16, name=f"mxc{k}",
                            tag=f"mxc{k}") for k in range(3)]

            # scratch target for the ACT-side pooled copy (only accum_out used)
            pscr = snp.tile([128, H * W // 2], BF16, name="pscr", tag="pscr")

            # warm-up operands: dependency-gated dummy matmuls keep the PE
            # HAM window busy right before the first real matmul so the real
            # stream starts at full clock (warm_x is touched from `prod` in
            # prep(0) to time the dummies against the routing chain)
            warm_w = snp.tile([128, 128], BF16, name="warm_w", tag="warm_w")
            nc.vector.memset(warm_w, 0.0)
            warm_x = snp.tile([128, NF], BF16, name="warm_x", tag="warm_x")
            nc.vector.memset(warm_x, 0.0)
            warm_ps0 = psp.tile([128, NF], F32, name="warm_ps0", tag="warmps",
                                bufs=1)
            for _ in range(36):
                nc.tensor.matmul(warm_ps0[:], lhsT=warm_w, rhs=warm_x,
                                 start=True, stop=True)

            def prep(s):
                """Routing + weight mixing for sample s (no PE involvement)."""
                i = s % 2
                pooled = [smp.tile([128, 1], F32, name=f"pool{s}_{t}",
                                   tag=f"pool{t}") for t in range(CT)]
                ph = smp.tile([128, 1], F32, name=f"ph{s}", tag="ph")
                # GAP: tile 0 on DVE; tile 1 split into a DVE half and an ACT
                # (Copy + accum_out) half so its reduction finishes ~2x sooner
                # after the tile-1 DMA lands
                nc.vector.reduce_sum(out=pooled[0], in_=xp[i][0][:, :, 1:W + 1],
                                     axis=AX.XY)
                nc.vector.reduce_sum(out=pooled[1],
                                     in_=xp[i][1][:, 0:H // 2, 1:W + 1],
                                     axis=AX.XY)
                nc.scalar.activation(out=pscr, in_=xp[i][1][:, H // 2:H, 1:W + 1],
                                     func=AF.Copy, accum_out=ph)
                prod = smp.tile([128, E], F32, name=f"prod{s}", tag="prod")
                nc.vector.tensor_scalar_mul(prod, rwt_sb[0], pooled[0])
                nc.vector.scalar_tensor_tensor(out=prod, in0=rwt_sb[1],
                                               scalar=pooled[1], in1=prod,
                                               op0=OP.mult, op1=OP.add)
                nc.vector.scalar_tensor_tensor(out=prod, in0=rwt_sb[1],
                                               scalar=ph, in1=prod,
                                               op0=OP.mult, op1=OP.add)
                if s == 0:
                    # touch warm_x from prod, then issue the warm-up matmuls:
                    # they run while the routing tail + mixing completes
                    nc.vector.tensor_copy(warm_x[0:1, 0:E], prod[0:1, :])
                    wps = psp.tile([128, NF], F32, name="warm_ps",
                                   tag="warmps", bufs=1)
                    for _ in range(18):
                        nc.tensor.matmul(wps[:], lhsT=warm_w, rhs=warm_x,
                                         start=True, stop=True)
                lg = smp.tile([128, E], F32, name=f"lg{s}", tag="lg")
                nc.gpsimd.partition_all_reduce(lg, prod, channels=128,
                                               reduce_op=bass_isa.ReduceOp.add)
                nc.vector.scalar_tensor_tensor(out=lg, in0=lg,
                                               scalar=1.0 / (H * W), in1=rb_bc,
                                               op0=OP.mult, op1=OP.add)
                rr = smp.tile([128, E], F32, name=f"rr{s}", tag="rr")
                nc.scalar.activation(out=rr, in_=lg, func=AF.Sigmoid, bias=zeros1)
                # mix per cout half: the first conv of the sample only waits
                # for the oi=0 half of the bank. cin tile 0 accumulates on
                # DVE; tile 1 gets its expert scaling from ACT (scaled Copy)
                # with DVE doing only the adds, so the two chains overlap.
                for oi in range(OTN):
                    # fp8-tap mix: tensor_scalar scalings into bf16 scratch,
                    # dense adds, single e4m3 quantization on the last add
                    nc.vector.tensor_scalar_mul(mxc[0][:, :], wq[0][:, oi],
                                                rr[:, 0:1])
                    nc.vector.tensor_scalar_mul(mxc[1][:, :], wq[1][:, oi],
                                                rr[:, 1:2])
                    nc.vector.tensor_scalar_mul(mxc[2][:, :], wq[2][:, oi],
                                                rr[:, 2:3])
                    nc.vector.tensor_add(mxc[0], mxc[0], mxc[1])
                    nc.vector.tensor_scalar_mul(mxc[1][:, :], wq[3][:, oi],
                                                rr[:, 3:4])
                    nc.vector.tensor_add(mxc[0], mxc[0], mxc[2])
                    nc.vector.tensor_add(mwq[i][:, oi], mxc[0], mxc[1])
                    nc.vector.tensor_scalar_mul(mw[i][0][:, oi],
                                                wb[0][0][:, oi], rr[:, 0:1])
                    for e in range(1, E):
                        nc.vector.scalar_tensor_tensor(
                            out=mw[i][0][:, oi], in0=wb[e][0][:, oi],
                            scalar=rr[:, e:e + 1], in1=mw[i][0][:, oi],
                            op0=OP.mult, op1=OP.add)
                    ce = [smp.tile([128, NB, 128], BF16, name=f"ce{s}_{oi}_{e}",
                                   tag=f"ce{e}", bufs=2) for e in range(E)]
                    for e in range(E):
                        nc.scalar.activation(out=ce[e], in_=wb[e][1][:, oi],
                                             func=AF.Copy, scale=rr[:, e:e + 1])
                    nc.vector.tensor_add(mw[i][1][:, oi], ce[0], ce[1])
                    nc.vector.tensor_add(mw[i][1][:, oi], mw[i][1][:, oi], ce[2])
                    nc.vector.tensor_add(mw[i][1][:, oi], mw[i][1][:, oi], ce[3])

            def conv(s, oi):
                """One output channel tile of sample s: matmuls + BN/ReLU +
                residual + store."""
                i = s % 2
                o0 = oi * 128
                n_mm = NQ + NB * CT
                for rc in range(RC):
                    r0 = rc * RH
                    acc = psp.tile([128, NF], F32, name=f"acc{s}_{oi}_{rc}",
                                   tag="acc")
                    k = 0
                    for q, (ki, kj) in enumerate(FP8_TAPS):
                        # DoubleRow: both cin tiles in one 256-deep fp8 MM
                        h_lo = max(r0, 1 - ki)
                        h_hi = min(r0 + RH - 1, H - ki)
                        nc.tensor.matmul(
                            acc[:, (h_lo - r0) * W:(h_hi - r0 + 1) * W],
                            lhsT=mwq[i][:, oi, q],
                            rhs=xq[i][:, :, h_lo + ki - 1:h_hi + ki,
                                      kj:kj + W],
                            start=(k == 0), stop=False, perf_mode=DR)
                        k += 1
                    for t in range(CT):
                        for q, (ki, kj) in enumerate(BF16_TAPS):
                            h_lo = max(r0, 1 - ki)
                            h_hi = min(r0 + RH - 1, H - ki)
                            k += 1
                            nc.tensor.matmul(
                                acc[:, (h_lo - r0) * W:(h_hi - r0 + 1) * W],
                                lhsT=mw[i][t][:, oi, q, :],
                                rhs=xp[i][t][:, h_lo + ki - 1:h_hi + ki,
                                             kj:kj + W],
                                start=False, stop=(k == n_mm))
                    ob = otp.tile([128, NF], BF16, name=f"ob{s}_{oi}_{rc}",
                                  tag="ob")
                    nc.scalar.activation(out=ob[:], in_=acc[:], func=AF.Relu,
                                         bias=bnb_sb[:, oi:oi + 1],
                                         scale=sc64)
                    ob3 = ob.rearrange("p (a b) -> p a b", a=RH)
                    nc.vector.tensor_add(ob3, ob3,
                                         xp[i][oi][:, r0:r0 + RH, 1:W + 1])
                    nc.sync.dma_start(out=y_ap[s, o0:o0 + 128, r0:r0 + RH, :],
                                      in_=ob3)

            prep(0)
            for s in range(BPC):
                if s + 1 < BPC:
                    j = (s + 1) % 2
                    for t in range(CT):
                        nc.sync.dma_start(
                            out=xp[j][t],
                            in_=x_ap[s + 1, t * 128:(t + 1) * 128, :, :])
                    nc.scalar.dma_start(out=xq[j], in_=xq_ap[s + 1])
                conv(s, 0)
                if s + 1 < BPC:
                    prep(s + 1)
                conv(s, 1)

    nc.compile()
    return nc


_CACHE = {}
_LOCK = threading.Lock()


def prepare_in_maps(inputs):
    """Host-side layout prep (sharding + transposes + dtype casts only)."""
    x = np.asarray(inputs["x"], dtype=np.float32)
    route_w = np.asarray(inputs["route_w"], dtype=np.float32)
    route_b = np.ascontiguousarray(np.asarray(inputs["route_b"], dtype=np.float32))
    expert_w = np.asarray(inputs["expert_w"], dtype=np.float32)
    bn_gamma = np.asarray(inputs["bn_gamma"], dtype=np.float32)
    bn_beta = np.asarray(inputs["bn_beta"], dtype=np.float32)
    bn_mean = np.asarray(inputs["bn_mean"], dtype=np.float32)
    bn_var = np.asarray(inputs["bn_var"], dtype=np.float32)

    # fold BN scale gamma' = gamma/sqrt(var+eps) into the expert bank (it
    # commutes with the linear routing mix); beta' = beta - mean*gamma' is
    # the only BN term left for the device (ACT Relu bias).
    inv = bn_gamma / np.sqrt(bn_var + BN_EPS)
    # WSC pre-scale keeps the e4m3-quantized mixed weights in normal range;
    # the ACT evacuation multiplies PSUM by 1/WSC before bias+ReLU.
    bank = expert_w * inv[None, :, None, None, None] * WSC
    cinv = bank.transpose(0, 2, 1, 3, 4)         # [E, CIN, COUT, ki, kj]
    wt = np.empty((E, CIN, OTN, NB, 128), np.float32)
    for q, (ki, kj) in enumerate(BF16_TAPS):
        wt[:, :, :, q] = cinv[:, :, :, ki, kj].reshape(E, CIN, OTN, 128)
    wt = np.ascontiguousarray(wt).astype(ml_dtypes.bfloat16)
    # DoubleRow bank [e, p, oi, q, j, o'] = bank[e, oi*128+o', 128j+p, ki, kj]
    wq = np.empty((E, 128, OTN, NQ, CT, 128), np.float32)
    for q, (ki, kj) in enumerate(FP8_TAPS):
        tq = cinv[:, :, :, ki, kj].reshape(E, CT, 128, OTN, 128)
        wq[:, :, :, q] = tq.transpose(0, 2, 3, 1, 4)
    wq = np.ascontiguousarray(wq).astype(ml_dtypes.bfloat16)
    rwt = np.ascontiguousarray(route_w.T)  # [CIN, E]
    bnb = np.ascontiguousarray(
        (bn_beta - bn_mean * inv).reshape(OTN, 128).T)  # [128, OTN]

    # width-pad on host: border columns arrive pre-zeroed, so the device DMA
    # is one fully contiguous transfer per (sample, cin-tile)
    xpad = np.zeros((B, CIN, H, WP), dtype=ml_dtypes.bfloat16)
    xpad[:, :, :, 1:W + 1] = x.astype(ml_dtypes.bfloat16)
    # fp8 image with the cin tiles paired along the DoubleRow k-tile dim:
    # xq[s, p, j, h, w] = e4m3(x[s, 128j+p, h, w])
    xq = np.zeros((B, 128, CT, H, WP), dtype=ml_dtypes.float8_e4m3)
    xq[:, :, :, :, 1:W + 1] = (x.reshape(B, CT, 128, H, W)
                               .transpose(0, 2, 1, 3, 4)
                               .astype(ml_dtypes.float8_e4m3))

    return [
        {"x": np.ascontiguousarray(xpad[c * BPC:(c + 1) * BPC]),
         "xq": np.ascontiguousarray(xq[c * BPC:(c + 1) * BPC]),
         "wt": wt, "wq": wq, "rwt": rwt, "rb": route_b, "bnb": bnb}
        for c in range(N_CORES)
    ]


def _get_nc():
    with _LOCK:
        if "nc" not in _CACHE:
            _CACHE["nc"] = build_bass()
        return _CACHE["nc"]


def kernel(**inputs):
    in_maps = prepare_in_maps(inputs)
    nc = _get_nc()
    res = run_bass_kernel_spmd(nc, in_maps, core_ids=list(range(N_CORES)))
    y = np.concatenate([np.asarray(r["y"]) for r in res.results], axis=0)
    return y.astype(np.float32)



# revision 24
# speedup vs baseline: 1.0413x; 1.0413x over previous
"""CondConv (routing -> per-sample mixed 3x3 conv -> frozen BN -> ReLU -> residual)
on 8 Trainium2 NeuronCores, data-parallel over batch (4 samples/core).

Per core:
  - expert bank resident in SBUF as bf16, host-pretransposed to
    [cin, cout-half, kk, 128] so each cout half is contiguous
  - routing: GAP (DVE reduce) -> dot with route_w (DVE + gpsimd partition
    all-reduce; keeps the PE queue free for conv matmuls) -> sigmoid (ACT)
  - per-sample mixed kernel: DVE scalar_tensor_tensor accumulation in bf16,
    split per cout half so the first conv starts after half the mixing
  - conv: per output tile, 12 accumulating matmuls per row chunk: 6 fp8e4
    DoubleRow taps (center+corners+top edge; both cin tiles in one 256-deep
    contraction at the cost of a single bf16 matmul) + 3 bf16 edge taps x 2
    cin tiles; fp32 PSUM; moving dim = 8 rows x 56 cols = 448
  - weights carry gamma'(BN)*64 from the host; ACT evacuation applies 1/64 +
    beta' bias + ReLU; residual add on DVE; bf16 output cast to fp32 on host
"""

import threading

import ml_dtypes
import numpy as np

import concourse.bass as bass
import concourse.mybir as mybir
import concourse.tile as tile
from concourse import bacc, bass_isa
from concourse.bass_utils import run_bass_kernel_spmd

F32 = mybir.dt.float32
BF16 = mybir.dt.bfloat16
FP8 = mybir.dt.float8e4
DR = mybir.MatmulPerfMode.DoubleRow
AX = mybir.AxisListType
OP = mybir.AluOpType
AF = mybir.ActivationFunctionType

N_CORES = 8
B, CIN, COUT, H, W, KS, E = 32, 256, 256, 56, 56, 3, 4
BPC = B // N_CORES  # samples per core
CT = CIN // 128     # cin partition tiles
OTN = COUT // 128   # cout partition tiles
KK = KS * KS
WP = W + 2          # width zero-padded (kj shifts); height handled by clipping
RC = 7              # row chunks per image
RH = H // RC        # rows per chunk
NF = RH * W         # moving-dim elements per matmul
BN_EPS = 1e-5
WSC = 64.0          # weight pre-scale for e4m3 range; 1/WSC on ACT evacuation

# tap split: center+corners run as fp8e4 DoubleRow matmuls (256-deep cin
# contraction, ~1.44x PE throughput); edges stay bf16. 5/9 of the contraction
# in fp8 keeps the measured pipeline rel err at ~1.8e-2 (< 2e-2 budget).
# Center first: it covers the full output chunk, so it carries start=True and
# clears every PSUM has_written bit; row-clipped taps then accumulate flat
# sub-slices (= 'same' padding semantics at top/bottom).
FP8_TAPS = [(1, 1), (0, 0), (0, 2), (2, 0), (2, 2), (0, 1)]
BF16_TAPS = [(1, 0), (1, 2), (2, 1)]
NQ = len(FP8_TAPS)
NB = len(BF16_TAPS)


def build_bass():
    nc = bacc.Bacc("TRN2", target_bir_lowering=False, debug=False)

    x_d = nc.dram_tensor("x", [BPC, CIN, H, WP], BF16, kind="ExternalInput")
    xq_d = nc.dram_tensor("xq", [BPC, 128, CT, H, WP], FP8,
                          kind="ExternalInput")
    wt_d = nc.dram_tensor("wt", [E, CIN, OTN, NB, 128], BF16,
                          kind="ExternalInput")
    wq_d = nc.dram_tensor("wq", [E, 128, OTN, NQ, CT, 128], BF16,
                          kind="ExternalInput")
    rwt_d = nc.dram_tensor("rwt", [CIN, E], F32, kind="ExternalInput")
    rb_d = nc.dram_tensor("rb", [E], F32, kind="ExternalInput")
    bnb_d = nc.dram_tensor("bnb", [128, OTN], F32, kind="ExternalInput")
    y_d = nc.dram_tensor("y", [BPC, COUT, H, W], BF16, kind="ExternalOutput")

    x_ap = x_d.ap()
    xq_ap = xq_d.ap()
    wq_ap = wq_d.ap()
    wt_ap = wt_d.ap()
    rwt_ap = rwt_d.ap()
    rb_ap = rb_d.ap()
    bnb_ap = bnb_d.ap()
    y_ap = y_d.ap()

    with tile.TileContext(nc) as tc:
        with (
            tc.tile_pool(name="wbp", bufs=1) as wbp,
            tc.tile_pool(name="xpp", bufs=1) as xpp,
            tc.tile_pool(name="mwp", bufs=1) as mwp,
            tc.tile_pool(name="otp", bufs=10) as otp,
            tc.tile_pool(name="snp", bufs=1) as snp,
            tc.tile_pool(name="smp", bufs=4) as smp,
            tc.tile_pool(name="psp", bufs=6, space="PSUM") as psp,
        ):
            # ---- persistent tiles ----
            wb = [[wbp.tile([128, OTN, NB, 128], BF16, name=f"wb{e}_{t}",
                            tag=f"wb{e}_{t}")
                   for t in range(CT)] for e in range(E)]
            wq = [wbp.tile([128, OTN, NQ, CT, 128], BF16, name=f"wq{e}",
                           tag=f"wq{e}") for e in range(E)]
            xp = [[xpp.tile([128, H, WP], BF16, name=f"xp{i}_{t}",
                            tag=f"xp{i}_{t}")
                   for t in range(CT)] for i in range(2)]
            xq = [xpp.tile([128, CT, H, WP], FP8, name=f"xq{i}", tag=f"xq{i}")
                  for i in range(2)]
            mw = [[mwp.tile([128, OTN, NB, 128], BF16, name=f"mw{i}_{t}",
                            tag=f"mw{i}_{t}")
                   for t in range(CT)] for i in range(2)]
            mwq = [mwp.tile([128, OTN, NQ, CT, 128], FP8, name=f"mwq{i}",
                            tag=f"mwq{i}") for i in range(2)]
            rwt_sb = [snp.tile([128, E], F32, name=f"rwt{t}", tag=f"rwt{t}")
                      for t in range(CT)]
            rb_bc = snp.tile([128, E], F32, name="rb_bc", tag="rb_bc")
            bnb_sb = snp.tile([128, OTN], F32, name="bnb", tag="bnb")

            # ---- preamble DMAs in priority order: queue order = bandwidth
            # priority. x(0) tiles split across two queues (routing critical
            # path), tiny params next, then the expert bank with the oi=0
            # halves first (mixing consumes them first).
            nc.sync.dma_start(out=xp[0][0][:, 0:28], in_=x_ap[0, 0:128, 0:28, :])
            nc.scalar.dma_start(out=xp[0][0][:, 28:56], in_=x_ap[0, 0:128, 28:56, :])
            nc.sync.dma_start(out=xp[0][1][:, 0:28], in_=x_ap[0, 128:256, 0:28, :])
            nc.gpsimd.dma_start(out=xp[0][1][:, 28:56], in_=x_ap[0, 128:256, 28:56, :])
            nc.sync.dma_start(out=xq[0][:, :, 0:28], in_=xq_ap[0, :, :, 0:28])
            nc.scalar.dma_start(out=xq[0][:, :, 28:56], in_=xq_ap[0, :, :, 28:56])
            for t in range(CT):
                nc.sync.dma_start(out=rwt_sb[t],
                                  in_=rwt_ap[t * 128:(t + 1) * 128, :])
            nc.sync.dma_start(
                out=rb_bc,
                in_=bass.AP(tensor=rb_ap.tensor, offset=0, ap=[[0, 128], [1, E]]))
            nc.sync.dma_start(out=bnb_sb, in_=bnb_ap[:, :])
            for e in range(E):
                nc.sync.dma_start(out=wq[e], in_=wq_ap[e])
            for oi in range(OTN):
                for e in range(E):
                    for t in range(CT):
                        nc.sync.dma_start(out=wb[e][t][:, oi],
                                          in_=wt_ap[e, t * 128:(t + 1) * 128, oi])

            # all-zeros per-partition scalar: explicit AP bias for ACT funcs
            # (the float-bias path needs a pre-registered const-AP database)
            zeros1 = snp.tile([128, 1], F32, name="zeros1", tag="zeros1")
            nc.vector.memset(zeros1, 0.0)
            sc64 = snp.tile([128, 1], F32, name="sc64", tag="sc64")
            nc.vector.memset(sc64, 1.0 / WSC)
            mxc = [snp.tile([128, NQ * CT * 128], BF16, name=f"mxc{k}",
                            tag=f"mxc{k}") for k in range(3)]

            # scratch target for the ACT-side pooled copy (only accum_out used)
            pscr = snp.tile([128, H * W // 2], BF16, name="pscr", tag="pscr")

            # warm-up operands: dependency-gated dummy matmuls keep the PE
            # HAM window busy right before the first real matmul so the real
            # stream starts at full clock (warm_x is touched from `prod` in
            # prep(0) to time the dummies against the routing chain)
            warm_w = snp.tile([128, 128], BF16, name="warm_w", tag="warm_w")
            nc.vector.memset(warm_w, 0.0)
            warm_x = snp.tile([128, NF], BF16, name="warm_x", tag="warm_x")
            nc.vector.memset(warm_x, 0.0)
            warm_ps0 = psp.tile([128, NF], F32, name="warm_ps0", tag="warmps",
                                bufs=1)
            for _ in range(48):
                nc.tensor.matmul(warm_ps0[:], lhsT=warm_w, rhs=warm_x,
                                 start=True, stop=True)

            def prep(s):
                """Routing + weight mixing for sample s (no PE involvement)."""
                i = s % 2
                pooled = [smp.tile([128, 1], F32, name=f"pool{s}_{t}",
                                   tag=f"pool{t}") for t in range(CT)]
                ph = smp.tile([128, 1], F32, name=f"ph{s}", tag="ph")
                # GAP: tile 0 on DVE; tile 1 split into a DVE half and an ACT
                # (Copy + accum_out) half so its reduction finishes ~2x sooner
                # after the tile-1 DMA lands
                nc.vector.reduce_sum(out=pooled[0], in_=xp[i][0][:, :, 1:W + 1],
                                     axis=AX.XY)
                nc.vector.reduce_sum(out=pooled[1],
                                     in_=xp[i][1][:, 0:H // 2, 1:W + 1],
                                     axis=AX.XY)
                nc.scalar.activation(out=pscr, in_=xp[i][1][:, H // 2:H, 1:W + 1],
                                     func=AF.Copy, accum_out=ph)
                prod = smp.tile([128, E], F32, name=f"prod{s}", tag="prod")
                nc.vector.tensor_scalar_mul(prod, rwt_sb[0], pooled[0])
                nc.vector.scalar_tensor_tensor(out=prod, in0=rwt_sb[1],
                                               scalar=pooled[1], in1=prod,
                                               op0=OP.mult, op1=OP.add)
                nc.vector.scalar_tensor_tensor(out=prod, in0=rwt_sb[1],
                                               scalar=ph, in1=prod,
                                               op0=OP.mult, op1=OP.add)
                if s == 0:
                    # touch warm_x from prod, then issue the warm-up matmuls:
                    # they run while the routing tail + mixing completes
                    nc.vector.tensor_copy(warm_x[0:1, 0:E], prod[0:1, :])
                    wps = psp.tile([128, NF], F32, name="warm_ps",
                                   tag="warmps", bufs=1)
                    for _ in range(18):
                        nc.tensor.matmul(wps[:], lhsT=warm_w, rhs=warm_x,
                                         start=True, stop=True)
                lg = smp.tile([128, E], F32, name=f"lg{s}", tag="lg")
                nc.gpsimd.partition_all_reduce(lg, prod, channels=128,
                                               reduce_op=bass_isa.ReduceOp.add)
                nc.vector.scalar_tensor_tensor(out=lg, in0=lg,
                                               scalar=1.0 / (H * W), in1=rb_bc,
                                               op0=OP.mult, op1=OP.add)
                rr = smp.tile([128, E], F32, name=f"rr{s}", tag="rr")
                nc.scalar.activation(out=rr, in_=lg, func=AF.Sigmoid, bias=zeros1)
                # mix per cout half: the first conv of the sample only waits
                # for the oi=0 half of the bank. cin tile 0 accumulates on
                # DVE; tile 1 gets its expert scaling from ACT (scaled Copy)
                # with DVE doing only the adds, so the two chains overlap.
                for oi in range(OTN):
                    # fp8-tap mix: tensor_scalar scalings into bf16 scratch,
                    # dense adds, single e4m3 quantization on the last add
                    nc.vector.tensor_scalar_mul(mxc[0][:, :], wq[0][:, oi],
                                                rr[:, 0:1])
                    nc.vector.tensor_scalar_mul(mxc[1][:, :], wq[1][:, oi],
                                                rr[:, 1:2])
                    nc.vector.tensor_scalar_mul(mxc[2][:, :], wq[2][:, oi],
                                                rr[:, 2:3])
                    nc.vector.tensor_add(mxc[0], mxc[0], mxc[1])
                    nc.vector.tensor_scalar_mul(mxc[1][:, :], wq[3][:, oi],
                                                rr[:, 3:4])
                    nc.vector.tensor_add(mxc[0], mxc[0], mxc[2])
                    nc.vector.tensor_add(mwq[i][:, oi], mxc[0], mxc[1])
                    nc.vector.tensor_scalar_mul(mw[i][0][:, oi],
                                                wb[0][0][:, oi], rr[:, 0:1])
                    for e in range(1, E):
                        nc.vector.scalar_tensor_tensor(
                            out=mw[i][0][:, oi], in0=wb[e][0][:, oi],
                            scalar=rr[:, e:e + 1], in1=mw[i][0][:, oi],
                            op0=OP.mult, op1=OP.add)
                    ce = [smp.tile([128, NB, 128], BF16, name=f"ce{s}_{oi}_{e}",
                                   tag=f"ce{e}", bufs=2) for e in range(E)]
                    for e in range(E):
                        nc.scalar.activation(out=ce[e], in_=wb[e][1][:, oi],
                                             func=AF.Copy, scale=rr[:, e:e + 1])
                    nc.vector.tensor_add(mw[i][1][:, oi], ce[0], ce[1])
                    nc.vector.tensor_add(mw[i][1][:, oi], mw[i][1][:, oi], ce[2])
                    nc.vector.tensor_add(mw[i][1][:, oi], mw[i][1][:, oi], ce[3])

            def conv(s, oi):
                """One output channel tile of sample s: matmuls + BN/ReLU +
                residual + store."""
                i = s % 2
                o0 = oi * 128
                n_mm = NQ + NB * CT
                for rc in range(RC):
                    r0 = rc * RH
                    acc = psp.tile([128, NF], F32, name=f"acc{s}_{oi}_{rc}",
                                   tag="acc")
                    k = 0
                    for q, (ki, kj) in enumerate(FP8_TAPS):
                        # DoubleRow: both cin tiles in one 256-deep fp8 MM
                        h_lo = max(r0, 1 - ki)
                        h_hi = min(r0 + RH - 1, H - ki)
                        nc.tensor.matmul(
                            acc[:, (h_lo - r0) * W:(h_hi - r0 + 1) * W],
                            lhsT=mwq[i][:, oi, q],
                            rhs=xq[i][:, :, h_lo + ki - 1:h_hi + ki,
                                      kj:kj + W],
                            start=(k == 0), stop=False, perf_mode=DR)
                        k += 1
                    for t in range(CT):
                        for q, (ki, kj) in enumerate(BF16_TAPS):
                            h_lo = max(r0, 1 - ki)
                            h_hi = min(r0 + RH - 1, H - ki)
                            k += 1
                            nc.tensor.matmul(
                                acc[:, (h_lo - r0) * W:(h_hi - r0 + 1) * W],
                                lhsT=mw[i][t][:, oi, q, :],
                                rhs=xp[i][t][:, h_lo + ki - 1:h_hi + ki,
                                             kj:kj + W],
                                start=False, stop=(k == n_mm))
                    ob = otp.tile([128, NF], BF16, name=f"ob{s}_{oi}_{rc}",
                                  tag="ob")
                    nc.scalar.activation(out=ob[:], in_=acc[:], func=AF.Relu,
                                         bias=bnb_sb[:, oi:oi + 1],
                                         scale=sc64)
                    ob3 = ob.rearrange("p (a b) -> p a b", a=RH)
                    nc.vector.tensor_add(ob3, ob3,
                                         xp[i][oi][:, r0:r0 + RH, 1:W + 1])
                    nc.sync.dma_start(out=y_ap[s, o0:o0 + 128, r0:r0 + RH, :],
                                      in_=ob3)

            prep(0)
            for s in range(BPC):
                if s + 1 < BPC:
                    j = (s + 1) % 2
                    for t in range(CT):
                        nc.sync.dma_start(
                            out=xp[j][t],
                            in_=x_ap[s + 1, t * 128:(t + 1) * 128, :, :])
                    nc.scalar.dma_start(out=xq[j], in_=xq_ap[s + 1])
                conv(s, 0)
                if s + 1 < BPC:
                    prep(s + 1)
                conv(s, 1)

    nc.compile()
    return nc


_CACHE = {}
_LOCK = threading.Lock()


def prepare_in_maps(inputs):
    """Host-side layout prep (sharding + transposes + dtype casts only)."""
    x = np.asarray(inputs["x"], dtype=np.float32)
    route_w = np.asarray(inputs["route_w"], dtype=np.float32)
    route_b = np.ascontiguousarray(np.asarray(inputs["route_b"], dtype=np.float32))
    expert_w = np.asarray(inputs["expert_w"], dtype=np.float32)
    bn_gamma = np.asarray(inputs["bn_gamma"], dtype=np.float32)
    bn_beta = np.asarray(inputs["bn_beta"], dtype=np.float32)
    bn_mean = np.asarray(inputs["bn_mean"], dtype=np.float32)
    bn_var = np.asarray(inputs["bn_var"], dtype=np.float32)

    # fold BN scale gamma' = gamma/sqrt(var+eps) into the expert bank (it
    # commutes with the linear routing mix); beta' = beta - mean*gamma' is
    # the only BN term left for the device (ACT Relu bias).
    inv = bn_gamma / np.sqrt(bn_var + BN_EPS)
    # WSC pre-scale keeps the e4m3-quantized mixed weights in normal range;
    # the ACT evacuation multiplies PSUM by 1/WSC before bias+ReLU.
    bank = expert_w * inv[None, :, None, None, None] * WSC
    cinv = bank.transpose(0, 2, 1, 3, 4)         # [E, CIN, COUT, ki, kj]
    wt = np.empty((E, CIN, OTN, NB, 128), np.float32)
    for q, (ki, kj) in enumerate(BF16_TAPS):
        wt[:, :, :, q] = cinv[:, :, :, ki, kj].reshape(E, CIN, OTN, 128)
    wt = np.ascontiguousarray(wt).astype(ml_dtypes.bfloat16)
    # DoubleRow bank [e, p, oi, q, j, o'] = bank[e, oi*128+o', 128j+p, ki, kj]
    wq = np.empty((E, 128, OTN, NQ, CT, 128), np.float32)
    for q, (ki, kj) in enumerate(FP8_TAPS):
        tq = cinv[:, :, :, ki, kj].reshape(E, CT, 128, OTN, 128)
        wq[:, :, :, q] = tq.transpose(0, 2, 3, 1, 4)
    wq = np.ascontiguousarray(wq).astype(ml_dtypes.bfloat16)
    rwt = np.ascontiguousarray(route_w.T)  # [CIN, E]
    bnb = np.ascontiguousarray(
        (bn_beta - bn_mean * inv).reshape(OTN, 128).T)  # [128, OTN]

    # width-pad on host: border columns arrive pre-zeroed, so the device DMA
    # is one fully contiguous transfer per (sample, cin-tile)
    xpad = np.zeros((B, CIN, H, WP), dtype=ml_dtypes.bfloat16)
    xpad[:, :, :, 1:W + 1] = x.astype(ml_dtypes.bfloat16)
    # fp8 image with the cin tiles paired along the DoubleRow k-tile dim:
    # xq[s, p, j, h, w] = e4m3(x[s, 128j+p, h, w])
    xq = np.zeros((B, 128, CT, H, WP), dtype=ml_dtypes.float8_e4m3)
    xq[:, :, :, :, 1:W + 1] = (x.reshape(B, CT, 128, H, W)
                               .transpose(0, 2, 1, 3, 4)
                               .astype(ml_dtypes.float8_e4m3))

    return [
        {"x": np.ascontiguousarray(xpad[c * BPC:(c + 1) * BPC]),
         "xq": np.ascontiguousarray(xq[c * BPC:(c + 1) * BPC]),
         "wt": wt, "wq": wq, "rwt": rwt, "rb": route_b, "bnb": bnb}
        for c in range(N_CORES)
    ]


def _get_nc():
    with _LOCK:
        if "nc" not in _CACHE:
            _CACHE["nc"] = build_bass()
        return _CACHE["nc"]


def kernel(**inputs):
    in_maps = prepare_in_maps(inputs)
    nc = _get_nc()
    res = run_bass_kernel_spmd(nc, in_maps, core_ids=list(range(N_CORES)))
    y = np.concatenate([np.asarray(r["y"]) for r in res.results], axis=0)
    return y.astype(np.float32)

